# revision 6
# baseline (speedup 1.0000x reference)
"""GRU encoder (embedding lookup + input GEMM + reset_after GRU scan) on 8
Trainium2 NeuronCores, data-parallel over batch (16 sequences/core).

Design notes:
- The recurrent matmul h@Wh streams Wh (12.6MB) through the PE each timestep;
  with float32r that is 1 cycle/row => ~10.2us/step lower bound. Batch size
  per core does not change this, so data-parallel sharding is free.
- Gate columns are pre-permuted host-side into [r0 z0 h0 r1 z1 h1] blocks of
  512 so per-gate elementwise work pipelines under the matmul stream.
- xg (input projection, incl. bias[0]) is precomputed into DRAM; during the
  scan it is injected into PSUM via an identity matmul ([I16; 1] stationary)
  which also adds bias[1] (row 16 of the xg tile), so PSUM accumulates
  xg + h@Wh + b2 with no extra vector work. For the candidate-gate tiles the
  stationary is [0; 1] so only b2 lands in PSUM (xh is added after r*rh).
- h is kept in both layouts: [16,1024] (batch-major, for elementwise+output)
  and transposed [128, 8*16] f32r chunks (PE transposes) for the next step's
  stationary operand.
"""
import numpy as np

import concourse.bacc as bacc
import concourse.mybir as mybir
import concourse.tile as tile
from concourse.bass import ds, IndirectOffsetOnAxis
from concourse.bass_utils import run_bass_kernel_spmd

dt = mybir.dt

# ---------------- configuration ----------------
NCORES = 8
B, T, E, U, VOCAB = 128, 256, 300, 1024, 50000
G = 3 * U                     # 3072 gate columns
BC = B // NCORES              # 16 sequences per core
KE = E + 1                    # kernel rows + bias[0] row
P = 128
NJ = G // 512                 # 6 gate tiles of 512
NU = U // P                   # 8 k-chunks of the recurrent contraction
UNROLL = 16                   # scan steps per For_i iteration


def gate_perm(u=U):
    """Column permutation: original [z r h] -> [r0 z0 h0 r1 z1 h1] halves."""
    half = u // 2
    blocks = []
    for h in range(2):
        blocks += [u + h * half,      # r
                   0 + h * half,      # z
                   2 * u + h * half]  # h
    return np.concatenate([np.arange(s, s + half) for s in blocks])


def build(nc, *, bc=BC, t_len=T, e=E, u=U, vocab=VOCAB, unroll=UNROLL):
    """Emit the full per-core program into nc. Returns nothing; tensors are
    found by name: inputs xi,etab,kaug,b2p,wh,cst,idf,zf; output oh."""
    g = 3 * u
    ke = e + 1
    nj = g // 512
    nu = u // P
    rows = bc * t_len
    mt = rows // P                      # phase-2 m-tiles
    n_iter = t_len // unroll
    e_chunks = []
    s = 0
    while s < ke:
        e_chunks.append((s, min(P, ke - s)))
        s += P

    xi = nc.dram_tensor("xi", [rows, 1], dt.int32, kind="ExternalInput")
    etab = nc.dram_tensor("etab", [vocab, e], dt.float32, kind="ExternalInput")
    kaug = nc.dram_tensor("kaug", [ke, g], dt.float32, kind="ExternalInput")
    b2p = nc.dram_tensor("b2p", [1, g], dt.float32, kind="ExternalInput")
    wh = nc.dram_tensor("wh", [u, g], dt.float32, kind="ExternalInput")
    cst = nc.dram_tensor("cst", [bc + 1, 2 * bc], dt.float32, kind="ExternalInput")
    idf = nc.dram_tensor("idf", [P, P], dt.float32, kind="ExternalInput")
    zf = nc.dram_tensor("zf", [P, nu * bc], dt.float32, kind="ExternalInput")
    oh = nc.dram_tensor("oh", [bc, t_len * u], dt.float32, kind="ExternalOutput")
    xgd = nc.dram_tensor("xgd", [rows, g], dt.float32r, kind="Internal")

    # persistent SBUF
    w_sb = [nc.alloc_sbuf_tensor(f"w_sb{c}", [P, g], dt.float32r) for c in range(nu)]
    xg_sb = [nc.alloc_sbuf_tensor(f"xg_sb{p}", [bc + 1, g], dt.float32r) for p in range(2)]
    h_sb = [nc.alloc_sbuf_tensor(f"h_sb{p}", [bc, u], dt.float32) for p in range(2)]
    hT_sb = [nc.alloc_sbuf_tensor(f"hT_sb{p}", [P, nu * bc], dt.float32r) for p in range(2)]
    cst_r = nc.alloc_sbuf_tensor("cst_r", [bc + 1, 2 * bc], dt.float32r)
    idf_sb = nc.alloc_sbuf_tensor("idf_sb", [P, P], dt.float32)

    # ---------------- phase 1+2: load weights, gather, input GEMM ----------
    with tile.TileContext(nc) as tc:
        for c in range(nu):
            nc.gpsimd.dma_start(w_sb[c].ap(), wh[c * P:(c + 1) * P, :])
        nc.gpsimd.dma_start(cst_r.ap(), cst[:])
        nc.sync.dma_start(idf_sb.ap(), idf[:])
        nc.gpsimd.dma_start(hT_sb[0].ap(), zf[:])
        nc.gpsimd.memset(h_sb[0].ap(), 0)
        for p in range(2):
            nc.gpsimd.dma_start(xg_sb[p].ap()[bc:bc + 1, :], b2p[:])

        with (
            tc.tile_pool(name="p2", bufs=1) as p2,
            tc.tile_pool(name="p2ps", bufs=1, space="PSUM") as p2ps,
        ):
            k_sb = [
                p2.tile([cn, g], dt.float32r, name=f"k_sb{i}")
                for i, (cs, cn) in enumerate(e_chunks)
            ]
            for i, (cs, cn) in enumerate(e_chunks):
                nc.gpsimd.dma_start(k_sb[i][:], kaug[cs:cs + cn, :])
            for m in range(mt):
                idx_t = p2.tile([P, 1], dt.int32, name=f"idx{m}", tag="idx", bufs=3)
                nc.sync.dma_start(idx_t[:], xi[m * P:(m + 1) * P, :])
                emb_t = p2.tile([P, ke], dt.float32, name=f"emb{m}", tag="emb", bufs=3)
                nc.gpsimd.memset(emb_t[:, e:ke], 1.0)
                nc.gpsimd.indirect_dma_start(
                    out=emb_t[:, 0:e],
                    out_offset=None,
                    in_=etab[:],
                    in_offset=IndirectOffsetOnAxis(ap=idx_t[:, :1], axis=0),
                )
                eT = []
                for i, (cs, cn) in enumerate(e_chunks):
                    pe_t = p2ps.tile([cn, P], dt.float32, name=f"peT{m}_{i}",
                                     tag="peT", bufs=2, space="PSUM")
                    nc.tensor.transpose(pe_t[:], emb_t[:, cs:cs + cn], idf_sb.ap())
                    eTi = p2.tile([cn, P], dt.float32r, name=f"eT{m}_{i}",
                                  tag=f"eT{i}", bufs=2)
                    nc.scalar.copy(eTi[:], pe_t[:])
                    eT.append(eTi)
                for j in range(nj):
                    ps_xg = p2ps.tile([P, 512], dt.float32, name=f"psxg{m}_{j}",
                                      tag="psxg", bufs=2, space="PSUM")
                    for i, (cs, cn) in enumerate(e_chunks):
                        nc.tensor.matmul(
                            ps_xg[:],
                            lhsT=eT[i][:],
                            rhs=k_sb[i][:, 512 * j:512 * (j + 1)],
                            start=(i == 0),
                            stop=(i == len(e_chunks) - 1),
                        )
                    xg_st = p2.tile([P, 512], dt.float32r, name=f"xgst{m}_{j}",
                                    tag="xgst", bufs=3)
                    if j % 2 == 0:
                        nc.scalar.copy(xg_st[:], ps_xg[:])
                    else:
                        nc.vector.tensor_copy(xg_st[:], ps_xg[:])
                    # m-tile rows are (b fixed, t = t0 + i); xgd rows are t*bc+b
                    b_idx, t0 = m // (t_len // P), (m % (t_len // P)) * P
                    dst = xgd[t0 * bc + b_idx: (t0 + P - 1) * bc + b_idx + 1: bc,
                              512 * j:512 * (j + 1)]
                    nc.sync.dma_start(dst, xg_st[:])

    # ---------------- phase 3: the scan ----------------
    with tile.TileContext(nc) as tc:
        with (
            tc.tile_pool(name="p3", bufs=1) as p3,
            tc.tile_pool(name="p3ps", bufs=1, space="PSUM") as p3ps,
        ):
            with tc.For_i(0, t_len, unroll) as iv:
                for s in range(unroll):
                    pp = s % 2
                    hp, hn = h_sb[pp], h_sb[1 - pp]
                    hTp, hTn = hT_sb[pp], hT_sb[1 - pp]
                    xg = xg_sb[pp]
                    nc.sync.dma_start(
                        xg.ap()[0:bc, :], xgd[ds(iv * bc + s * bc, bc), :]
                    )
                    r_t = [None, None]
                    z_t = [None, None]
                    for j in range(nj):
                        half, gate = j // 3, j % 3
                        co = 512 * j
                        rg = p3ps.tile([bc, 512], dt.float32, name=f"rg{s}_{j}",
                                       tag=f"rg{j}", space="PSUM")
                        nc.tensor.matmul(
                            rg[:],
                            lhsT=cst_r.ap()[:, (bc if gate == 2 else 0):(2 * bc if gate == 2 else bc)],
                            rhs=xg.ap()[:, co:co + 512],
                            start=True, stop=False,
                        )
                        for c in range(nu):
                            nc.tensor.matmul(
                                rg[:],
                                lhsT=hTp.ap()[:, bc * c:bc * (c + 1)],
                                rhs=w_sb[c].ap()[:, co:co + 512],
                                start=False, stop=(c == nu - 1),
                            )
                        if gate == 0:
                            r_t[half] = p3.tile([bc, 512], dt.float32,
                                                name=f"r{s}_{half}", tag=f"r{half}", bufs=2)
                            nc.scalar.activation(
                                r_t[half][:], rg[:], mybir.ActivationFunctionType.Sigmoid)
                        elif gate == 1:
                            z_t[half] = p3.tile([bc, 512], dt.float32,
                                                name=f"z{s}_{half}", tag=f"z{half}", bufs=2)
                            nc.scalar.activation(
                                z_t[half][:], rg[:], mybir.ActivationFunctionType.Sigmoid)
                        else:
                            uo = 512 * half
                            tm = p3.tile([bc, 512], dt.float32, name=f"tm{s}_{half}",
                                         tag="tm", bufs=2)
                            nc.vector.tensor_mul(tm[:], r_t[half][:], rg[:])
                            nc.vector.tensor_add(
                                tm[:], tm[:], xg.ap()[0:bc, co:co + 512].bitcast(dt.float32))
                            hh = p3.tile([bc, 512], dt.float32, name=f"hh{s}_{half}",
                                         tag="hh", bufs=2)
                            nc.scalar.activation(
                                hh[:], tm[:], mybir.ActivationFunctionType.Tanh)
                            # h' = hh + z*(h - hh)
                            d_t = p3.tile([bc, 512], dt.float32, name=f"d{s}_{half}",
                                          tag="dt", bufs=2)
                            nc.vector.tensor_sub(d_t[:], hp.ap()[:, uo:uo + 512], hh[:])
                            nc.vector.tensor_mul(d_t[:], z_t[half][:], d_t[:])
                            nc.vector.tensor_add(hn.ap()[:, uo:uo + 512], hh[:], d_t[:])
                            for cl in range(4):
                                c = 4 * half + cl
                                pt = p3ps.tile([P, bc], dt.float32, name=f"pt{s}_{c}",
                                               tag="pt", bufs=2, space="PSUM")
                                nc.tensor.transpose(
                                    pt[:], hn.ap()[:, P * c:P * (c + 1)],
                                    idf_sb.ap()[0:bc, 0:bc])
                                nc.scalar.copy(hTn.ap()[:, bc * c:bc * (c + 1)], pt[:])
                    nc.sync.dma_start(
                        oh[:, ds(iv * u + s * u, u)], hn.ap()[:]
                    )


_BUILT = {}


def _get_nc():
    if "nc" not in _BUILT:
        nc = bacc.Bacc("TRN2", target_bir_lowering=False, debug=False,
                       num_devices=NCORES)
        build(nc)
        nc.compile()
        _BUILT["nc"] = nc
    return _BUILT["nc"]


def make_inputs(x, embedding, kernel, recurrent_kernel, bias, *, ncores=NCORES):
    """Host-side prep: returns list of per-core input dicts."""
    perm = gate_perm(U)
    kaug = np.concatenate(
        [np.asarray(kernel, np.float32), np.asarray(bias[0], np.float32)[None, :]], 0
    )[:, perm].copy()
    b2p = np.asarray(bias[1], np.float32)[perm][None, :].copy()
    whp = np.asarray(recurrent_kernel, np.float32)[:, perm].copy()
    cstv = np.zeros((BC + 1, 2 * BC), np.float32)
    cstv[:BC, :BC] = np.eye(BC)
    cstv[BC, :] = 1.0
    idfv = np.eye(P, dtype=np.float32)
    zfv = np.zeros((P, NU * BC), np.float32)
    etabv = np.ascontiguousarray(np.asarray(embedding, np.float32))
    xall = np.asarray(x, np.int32)
    maps = []
    for c in range(ncores):
        xi = xall[c * BC:(c + 1) * BC, :].reshape(-1, 1).astype(np.int32)
        maps.append({
            "xi": np.ascontiguousarray(xi),
            "etab": etabv,
            "kaug": kaug,
            "b2p": b2p,
            "wh": whp,
            "cst": cstv,
            "idf": idfv,
            "zf": zfv,
        })
    return maps


def assemble(results):
    """results: list of per-core dicts with 'oh' -> (output, state)."""
    outs = [r["oh"].reshape(BC, T, U) for r in results]
    output = np.concatenate(outs, axis=0)
    return output, output[:, -1, :].copy()


def kernel(x, embedding, kernel, recurrent_kernel, bias):
    nc = _get_nc()
    in_maps = make_inputs(x, embedding, kernel, recurrent_kernel, bias)
    res = run_bass_kernel_spmd(nc, in_maps, core_ids=list(range(NCORES)))
    return assemble(res.results)


# revision 7
# speedup vs baseline: 1.3646x; 1.3646x over previous
"""GRU encoder (embedding lookup + input GEMM + reset_after GRU scan) on 8
Trainium2 NeuronCores, data-parallel over batch (16 sequences/core).

Design notes:
- The recurrent matmul h@Wh streams Wh (12.6MB) through the PE each timestep;
  with float32r that is 1 cycle/row => ~10.2us/step lower bound. Batch size
  per core does not change this, so data-parallel sharding is free.
- Gate columns are pre-permuted host-side into [r0 z0 h0 r1 z1 h1] blocks of
  512 so per-gate elementwise work pipelines under the matmul stream.
- xg (input projection, incl. bias[0]) is precomputed into DRAM; during the
  scan it is injected into PSUM via an identity matmul ([I16; 1] stationary)
  which also adds bias[1] (row 16 of the xg tile), so PSUM accumulates
  xg + h@Wh + b2 with no extra vector work. For the candidate-gate tiles the
  stationary is [0; 1] so only b2 lands in PSUM (xh is added after r*rh).
- h is kept in both layouts: [16,1024] (batch-major, for elementwise+output)
  and transposed [128, 8*16] f32r chunks (PE transposes) for the next step's
  stationary operand.
"""
import numpy as np

import concourse.bacc as bacc
import concourse.mybir as mybir
import concourse.tile as tile
from concourse.bass import ds, IndirectOffsetOnAxis
from concourse.bass_utils import run_bass_kernel_spmd

dt = mybir.dt

# ---------------- configuration ----------------
NCORES = 8
B, T, E, U, VOCAB = 128, 256, 300, 1024, 50000
G = 3 * U                     # 3072 gate columns
BC = B // NCORES              # 16 sequences per core
KE = E + 1                    # kernel rows + bias[0] row
P = 128
NJ = G // 512                 # 6 gate tiles of 512
NU = U // P                   # 8 k-chunks of the recurrent contraction
UNROLL = 16                   # scan steps per For_i iteration
W_DT = dt.bfloat16            # recurrent weight/state matmul dtype (vs float32r)


def gate_perm(u=U):
    """Column permutation: original [z r h] -> [r0 z0 h0 r1 z1 h1] halves."""
    half = u // 2
    blocks = []
    for h in range(2):
        blocks += [u + h * half,      # r
                   0 + h * half,      # z
                   2 * u + h * half]  # h
    return np.concatenate([np.arange(s, s + half) for s in blocks])


def build(nc, *, bc=BC, t_len=T, e=E, u=U, vocab=VOCAB, unroll=UNROLL):
    """Emit the full per-core program into nc. Returns nothing; tensors are
    found by name: inputs xi,etab,kaug,b2p,wh,cst,idf,zf; output oh."""
    g = 3 * u
    ke = e + 1
    nj = g // 512
    nu = u // P
    rows = bc * t_len
    mt = rows // P                      # phase-2 m-tiles
    n_iter = t_len // unroll
    e_chunks = []
    s = 0
    while s < ke:
        e_chunks.append((s, min(P, ke - s)))
        s += P

    xi = nc.dram_tensor("xi", [rows, 1], dt.int32, kind="ExternalInput")
    etab = nc.dram_tensor("etab", [vocab, e], dt.float32, kind="ExternalInput")
    kaug = nc.dram_tensor("kaug", [ke, g], dt.float32, kind="ExternalInput")
    b2p = nc.dram_tensor("b2p", [1, g], dt.float32, kind="ExternalInput")
    wh = nc.dram_tensor("wh", [u, g], dt.float32, kind="ExternalInput")
    cst = nc.dram_tensor("cst", [bc + 1, 2 * bc], dt.float32, kind="ExternalInput")
    idf = nc.dram_tensor("idf", [P, P], dt.float32, kind="ExternalInput")
    zf = nc.dram_tensor("zf", [P, nu * bc], dt.float32, kind="ExternalInput")
    oh = nc.dram_tensor("oh", [bc, t_len * u], dt.float32, kind="ExternalOutput")
    xgd = nc.dram_tensor("xgd", [rows, g], dt.float32r, kind="Internal")

    # persistent SBUF
    w_sb = [nc.alloc_sbuf_tensor(f"w_sb{c}", [P, g], W_DT) for c in range(nu)]
    xg_sb = [nc.alloc_sbuf_tensor(f"xg_sb{p}", [bc + 1, g], dt.float32r) for p in range(2)]
    h_sb = [nc.alloc_sbuf_tensor(f"h_sb{p}", [bc, u], dt.float32) for p in range(2)]
    hT_sb = [nc.alloc_sbuf_tensor(f"hT_sb{p}", [P, nu * bc], W_DT) for p in range(2)]
    cst_r = nc.alloc_sbuf_tensor("cst_r", [bc + 1, 2 * bc], dt.float32r)
    idf_sb = nc.alloc_sbuf_tensor("idf_sb", [P, P], dt.float32)

    # ---------------- phase 1+2: load weights, gather, input GEMM ----------
    with tile.TileContext(nc) as tc:
        for c in range(nu):
            nc.gpsimd.dma_start(w_sb[c].ap(), wh[c * P:(c + 1) * P, :])
        nc.gpsimd.dma_start(cst_r.ap(), cst[:])
        nc.sync.dma_start(idf_sb.ap(), idf[:])
        nc.gpsimd.dma_start(hT_sb[0].ap(), zf[:])
        nc.gpsimd.memset(h_sb[0].ap(), 0)
        for p in range(2):
            nc.gpsimd.dma_start(xg_sb[p].ap()[bc:bc + 1, :], b2p[:])

        with (
            tc.tile_pool(name="p2", bufs=1) as p2,
            tc.tile_pool(name="p2ps", bufs=1, space="PSUM") as p2ps,
        ):
            k_sb = [
                p2.tile([cn, g], dt.float32r, name=f"k_sb{i}")
                for i, (cs, cn) in enumerate(e_chunks)
            ]
            for i, (cs, cn) in enumerate(e_chunks):
                nc.gpsimd.dma_start(k_sb[i][:], kaug[cs:cs + cn, :])
            for m in range(mt):
                idx_t = p2.tile([P, 1], dt.int32, name=f"idx{m}", tag="idx", bufs=3)
                nc.sync.dma_start(idx_t[:], xi[m * P:(m + 1) * P, :])
                emb_t = p2.tile([P, ke], dt.float32, name=f"emb{m}", tag="emb", bufs=3)
                nc.gpsimd.memset(emb_t[:, e:ke], 1.0)
                nc.gpsimd.indirect_dma_start(
                    out=emb_t[:, 0:e],
                    out_offset=None,
                    in_=etab[:],
                    in_offset=IndirectOffsetOnAxis(ap=idx_t[:, :1], axis=0),
                )
                eT = []
                for i, (cs, cn) in enumerate(e_chunks):
                    pe_t = p2ps.tile([cn, P], dt.float32, name=f"peT{m}_{i}",
                                     tag="peT", bufs=2, space="PSUM")
                    nc.tensor.transpose(pe_t[:], emb_t[:, cs:cs + cn], idf_sb.ap())
                    eTi = p2.tile([cn, P], dt.float32r, name=f"eT{m}_{i}",
                                  tag=f"eT{i}", bufs=2)
                    nc.scalar.copy(eTi[:], pe_t[:])
                    eT.append(eTi)
                for j in range(nj):
                    ps_xg = p2ps.tile([P, 512], dt.float32, name=f"psxg{m}_{j}",
                                      tag="psxg", bufs=2, space="PSUM")
                    for i, (cs, cn) in enumerate(e_chunks):
                        nc.tensor.matmul(
                            ps_xg[:],
                            lhsT=eT[i][:],
                            rhs=k_sb[i][:, 512 * j:512 * (j + 1)],
                            start=(i == 0),
                            stop=(i == len(e_chunks) - 1),
                        )
                    xg_st = p2.tile([P, 512], dt.float32r, name=f"xgst{m}_{j}",
                                    tag="xgst", bufs=3)
                    if j % 2 == 0:
                        nc.scalar.copy(xg_st[:], ps_xg[:])
                    else:
                        nc.vector.tensor_copy(xg_st[:], ps_xg[:])
                    # m-tile rows are (b fixed, t = t0 + i); xgd rows are t*bc+b
                    b_idx, t0 = m // (t_len // P), (m % (t_len // P)) * P
                    dst = xgd[t0 * bc + b_idx: (t0 + P - 1) * bc + b_idx + 1: bc,
                              512 * j:512 * (j + 1)]
                    nc.sync.dma_start(dst, xg_st[:])

    # ---------------- phase 3: the scan ----------------
    with tile.TileContext(nc) as tc:
        with (
            tc.tile_pool(name="p3", bufs=1) as p3,
            tc.tile_pool(name="p3ps", bufs=1, space="PSUM") as p3ps,
        ):
            with tc.For_i(0, t_len, unroll) as iv:
                for s in range(unroll):
                    pp = s % 2
                    hp, hn = h_sb[pp], h_sb[1 - pp]
                    hTp, hTn = hT_sb[pp], hT_sb[1 - pp]
                    xg = xg_sb[pp]
                    nc.sync.dma_start(
                        xg.ap()[0:bc, :], xgd[ds(iv * bc + s * bc, bc), :]
                    )
                    r_t = [None, None]
                    z_t = [None, None]
                    for j in range(nj):
                        half, gate = j // 3, j % 3
                        co = 512 * j
                        rg = p3ps.tile([bc, 512], dt.float32, name=f"rg{s}_{j}",
                                       tag=f"rg{j}", space="PSUM")
                        nc.tensor.matmul(
                            rg[:],
                            lhsT=cst_r.ap()[:, (bc if gate == 2 else 0):(2 * bc if gate == 2 else bc)],
                            rhs=xg.ap()[:, co:co + 512],
                            start=True, stop=False,
                        )
                        for c in range(nu):
                            nc.tensor.matmul(
                                rg[:],
                                lhsT=hTp.ap()[:, bc * c:bc * (c + 1)],
                                rhs=w_sb[c].ap()[:, co:co + 512],
                                start=False, stop=(c == nu - 1),
                            )
                        if gate == 0:
                            r_t[half] = p3.tile([bc, 512], dt.float32,
                                                name=f"r{s}_{half}", tag=f"r{half}", bufs=2)
                            nc.scalar.activation(
                                r_t[half][:], rg[:], mybir.ActivationFunctionType.Sigmoid)
                        elif gate == 1:
                            z_t[half] = p3.tile([bc, 512], dt.float32,
                                                name=f"z{s}_{half}", tag=f"z{half}", bufs=2)
                            nc.scalar.activation(
                                z_t[half][:], rg[:], mybir.ActivationFunctionType.Sigmoid)
                        else:
                            uo = 512 * half
                            tm = p3.tile([bc, 512], dt.float32, name=f"tm{s}_{half}",
                                         tag="tm", bufs=2)
                            nc.vector.tensor_mul(tm[:], r_t[half][:], rg[:])
                            nc.vector.tensor_add(
                                tm[:], tm[:], xg.ap()[0:bc, co:co + 512].bitcast(dt.float32))
                            hh = p3.tile([bc, 512], dt.float32, name=f"hh{s}_{half}",
                                         tag="hh", bufs=2)
                            nc.scalar.activation(
                                hh[:], tm[:], mybir.ActivationFunctionType.Tanh)
                            # h' = hh + z*(h - hh)
                            d_t = p3.tile([bc, 512], dt.float32, name=f"d{s}_{half}",
                                          tag="dt", bufs=2)
                            nc.vector.tensor_sub(d_t[:], hp.ap()[:, uo:uo + 512], hh[:])
                            nc.vector.tensor_mul(d_t[:], z_t[half][:], d_t[:])
                            nc.vector.tensor_add(hn.ap()[:, uo:uo + 512], hh[:], d_t[:])
                            for cl in range(4):
                                c = 4 * half + cl
                                pt = p3ps.tile([P, bc], dt.float32, name=f"pt{s}_{c}",
                                               tag="pt", bufs=2, space="PSUM")
                                nc.tensor.transpose(
                                    pt[:], hn.ap()[:, P * c:P * (c + 1)],
                                    idf_sb.ap()[0:bc, 0:bc])
                                nc.scalar.copy(hTn.ap()[:, bc * c:bc * (c + 1)], pt[:])
                    nc.sync.dma_start(
                        oh[:, ds(iv * u + s * u, u)], hn.ap()[:]
                    )


_BUILT = {}


def _get_nc():
    if "nc" not in _BUILT:
        nc = bacc.Bacc("TRN2", target_bir_lowering=False, debug=False,
                       num_devices=NCORES)
        build(nc)
        nc.compile()
        _BUILT["nc"] = nc
    return _BUILT["nc"]


def make_inputs(x, embedding, kernel, recurrent_kernel, bias, *, ncores=NCORES):
    """Host-side prep: returns list of per-core input dicts."""
    perm = gate_perm(U)
    kaug = np.concatenate(
        [np.asarray(kernel, np.float32), np.asarray(bias[0], np.float32)[None, :]], 0
    )[:, perm].copy()
    b2p = np.asarray(bias[1], np.float32)[perm][None, :].copy()
    whp = np.asarray(recurrent_kernel, np.float32)[:, perm].copy()
    cstv = np.zeros((BC + 1, 2 * BC), np.float32)
    cstv[:BC, :BC] = np.eye(BC)
    cstv[BC, :] = 1.0
    idfv = np.eye(P, dtype=np.float32)
    zfv = np.zeros((P, NU * BC), np.float32)
    etabv = np.ascontiguousarray(np.asarray(embedding, np.float32))
    xall = np.asarray(x, np.int32)
    maps = []
    for c in range(ncores):
        xi = xall[c * BC:(c + 1) * BC, :].reshape(-1, 1).astype(np.int32)
        maps.append({
            "xi": np.ascontiguousarray(xi),
            "etab": etabv,
            "kaug": kaug,
            "b2p": b2p,
            "wh": whp,
            "cst": cstv,
            "idf": idfv,
            "zf": zfv,
        })
    return maps


def assemble(results):
    """results: list of per-core dicts with 'oh' -> (output, state)."""
    outs = [r["oh"].reshape(BC, T, U) for r in results]
    output = np.concatenate(outs, axis=0)
    return output, output[:, -1, :].copy()


def kernel(x, embedding, kernel, recurrent_kernel, bias):
    nc = _get_nc()
    in_maps = make_inputs(x, embedding, kernel, recurrent_kernel, bias)
    res = run_bass_kernel_spmd(nc, in_maps, core_ids=list(range(NCORES)))
    return assemble(res.results)
